# revision 29
# baseline (speedup 1.0000x reference)
"""Batched multi-head attention (32 heads, S=2048, D=128, fp32) on 8 Trainium2
NeuronCores.

Sharding: head-parallel — core i computes heads [4i, 4i+4) independently
(no collectives), takes full fp32 inputs, returns the full fp32 output.

Per-core kernel design (4 heads):
  - Q, K are cast fp32->fp16 by a SWDGE (gpsimd) DMA into DRAM staging, then
    xbar DMA-transposed into SBUF as QT/KT [d=128, s=2048] (contraction dim on
    partitions). V is cast-DMA'd natively into SBUF tiles [sk=128, 129] with a
    ones column appended (col 128) so the PV matmul also produces the softmax
    denominator.
  - For each head, for each of 16 sk-tiles:
      * scores^T tile  [sk=128, q=2048] = (K_tile @ Q^T) via 4 matmuls N=512
        (lhsT = KT tile stationary, rhs = QT moving) into PSUM [128,512] slots
      * DVE copies PSUM -> fp32 SBUF staging, one ACT exp over [128, 2048]
        (scale = 1/sqrt(128) folded into the activation's free affine) writes
        P^T fp16. No max-subtraction: scores*scale ~ N(0,1), exp is safe.
      * 16 PV matmuls N=129 (lhsT = P^T slice [128,128] stationary,
        rhs = V_aug tile [128,129] moving) accumulate into 6 packed PSUM
        tiles [128, 387] (3 sq-subtiles per PSUM bank).
  - Normalize: DVE reciprocal of the sums column, tensor_scalar multiply,
    DMA out fp32 [128,128] row-tiles.
"""

import os
import numpy as np

BH, S, D = 32, 2048, 128
N_CORES = 8
HPC = BH // N_CORES  # heads per core
SK = S // 128  # sk tiles per head
SQ = S // 128  # sq subtiles per head
SCALE = 1.0 / float(np.sqrt(D))

_CACHE = {}


def _install_ntff_hook():
    """Provide antenv.axon_hooks (absent in this container) so that
    run_bass_kernel_spmd(trace=True) can capture NTFF profiles."""
    import contextlib, ctypes, sys, types

    if "antenv.axon_hooks" in sys.modules:
        return
    so_path = "/opt/axon/libaxon_pjrt.so"
    hook = None
    try:
        lib = ctypes.CDLL(so_path)
        if hasattr(lib, "axon_start_nrt_profile"):
            lib.axon_start_nrt_profile.argtypes = [
                ctypes.POINTER(ctypes.c_int64),
                ctypes.c_size_t,
            ]
            lib.axon_start_nrt_profile.restype = ctypes.c_int64
            lib.axon_stop_nrt_profile.argtypes = [ctypes.c_char_p]
            lib.axon_stop_nrt_profile.restype = ctypes.c_int64

            @contextlib.contextmanager
            def _h(output_dir, device_ids):
                import jax

                jax.devices()
                if device_ids:
                    ids = (ctypes.c_int64 * len(device_ids))(*device_ids)
                    rc = lib.axon_start_nrt_profile(ids, len(device_ids))
                else:
                    rc = lib.axon_start_nrt_profile(None, 0)
                if rc != 0:
                    raise RuntimeError(f"axon_start_nrt_profile rc={rc}")
                try:
                    yield
                finally:
                    n = lib.axon_stop_nrt_profile(str(output_dir).encode())
                    print(f"ntff profile: {n} file(s) in {output_dir}")

            hook = _h
    except OSError:
        pass
    mod = types.ModuleType("antenv.axon_hooks")
    mod.get_axon_ntff_profile_hook = lambda: hook
    mod.set_axon_ntff_profile_hook = lambda h: None
    sys.modules["antenv.axon_hooks"] = mod


def _split_sync_waits(nc, maxw=1):
    """The walrus codegen in this container rejects instructions carrying more
    than `maxw` sync waits (Tile's scheduler can attach several). Move the
    excess waits onto same-engine nop instructions inserted just before."""
    from concourse import mybir

    n_split = 0
    for f in nc.m.functions:
        for bb in f.blocks:
            out = []
            for inst in bb.instructions:
                si = inst.sync_info
                if si is not None and si.on_wait and len(si.on_wait) > maxw:
                    waits = list(si.on_wait)
                    carriers, keep = waits[:-maxw], waits[-maxw:]
                    si.on_wait = keep
                    inst.sync_info = si
                    for i in range(0, len(carriers), maxw):
                        n_split += 1
                        nop = mybir.InstNoOp(
                            name=f"{inst.name}_wsplit{i}", ins=[], outs=[]
                        )
                        nop.engine = inst.engine
                        nop.sync_info = mybir.SyncInfo(
                            on_wait=carriers[i : i + maxw], on_update=[]
                        )
                        if hasattr(nc, "inst_map"):
                            nc.inst_map[nop.name] = nop
                        out.append(nop)
                out.append(inst)
            bb.instructions[:] = out
    return n_split


def _build():
    import concourse.bass as bass
    from concourse import mybir
    import concourse.tile as tile
    from concourse.masks import make_identity

    fp16 = mybir.dt.float16
    fp32 = mybir.dt.float32
    AF = mybir.ActivationFunctionType

    nc = bass.Bass("TRN2", target_bir_lowering=False, debug=False)
    q = nc.dram_tensor("q", [HPC, S, D], fp32, kind="ExternalInput").ap()
    k = nc.dram_tensor("k", [HPC, S, D], fp32, kind="ExternalInput").ap()
    v = nc.dram_tensor("v", [HPC, S, D], fp32, kind="ExternalInput").ap()
    o = nc.dram_tensor("o", [HPC, S, D], fp32, kind="ExternalOutput").ap()

    with tile.TileContext(nc) as tc:
        with (
            tc.tile_pool(name="ident", bufs=1) as ident_pool,
            tc.tile_pool(name="native", bufs=4) as native_pool,
            tc.tile_pool(name="qt", bufs=HPC) as qt_pool,
            tc.tile_pool(name="kt", bufs=HPC) as kt_pool,
            tc.tile_pool(name="vsb", bufs=HPC) as v_pool,
            tc.tile_pool(name="pt", bufs=3) as pt_pool,
            tc.tile_pool(name="psum_s", bufs=2, space="PSUM") as psum_s_pool,
            tc.tile_pool(name="psum_o", bufs=4, space="PSUM") as psum_o_pool,
            tc.tile_pool(name="outsb", bufs=6) as out_pool,
            tc.tile_pool(name="norm", bufs=4) as norm_pool,
        ):
            # ---- input prep: NO DMA-transposes (Tile globally serializes
            # every DMA against any in-flight xbar transpose, which makes
            # input prep a ~110us serial chain). Instead: SWDGE cast-DMA to
            # SBUF in the native [s,d] block layout, then transpose each
            # [128,128] block on the PE with a plain matmul against an fp16
            # identity, DVE-copying PSUM -> qt/kt. Transposes borrow psum_s
            # slots; for head h+1 they are emitted mid-way through head h's
            # second chunk so their slot claims sit behind the critical
            # QK/exp traffic but complete before head h+1 begins.
            ident = ident_pool.tile([128, 128], fp16)
            make_identity(nc, ident[:])

            qts, kts, vsbs, nats = {}, {}, {}, {}

            def cast_inputs(h):
                if h >= HPC:
                    return
                qn = native_pool.tile([128, S], fp16, tag="nat", name=f"qn_{h}")
                nc.gpsimd.dma_start(
                    qn[:].rearrange("p (t d) -> p t d", d=D),
                    q[h].rearrange("(t p) d -> p t d", p=128),
                )
                kn = native_pool.tile([128, S], fp16, tag="nat", name=f"kn_{h}")
                nc.gpsimd.dma_start(
                    kn[:].rearrange("p (t d) -> p t d", d=D),
                    k[h].rearrange("(t p) d -> p t d", p=128),
                )
                nats[h] = (qn, kn)
                vsb = v_pool.tile([128, SK * 129], fp16, tag="vsb", name=f"vsb_{h}")
                nc.gpsimd.memset(vsb[:], 1.0)
                vv = vsb[:].rearrange("p (t c) -> p t c", c=129)
                nc.gpsimd.dma_start(
                    vv[:, :, 0:D], v[h].rearrange("(t p) d -> p t d", p=128)
                )
                vsbs[h] = vsb

            def pe_transpose_part(nat, out, g):
                slot = psum_s_pool.tile([128, 1024], fp32, tag="ps")
                for t in range(8):
                    blk = (g * 8 + t) * 128
                    nc.tensor.matmul(
                        slot[:, t * 128 : (t + 1) * 128],
                        nat[:, blk : blk + 128],
                        ident[:],
                        start=(t % 4 == 0),
                        stop=True,
                        skip_group_check=True,
                    )
                nc.vector.tensor_copy(out[:, g * 1024 : (g + 1) * 1024], slot[:])

            def xpose_part(h, idx):
                """idx 0,1 -> q halves; 2,3 -> k halves. One psum slot-use
                each, so the bursts can be spread between QK/exp traffic."""
                if h >= HPC:
                    return
                qn, kn = nats[h]
                if idx == 0:
                    qts[h] = qt_pool.tile([128, S], fp16, tag="qt", name=f"qt_{h}")
                if idx == 2:
                    kts[h] = kt_pool.tile([128, S], fp16, tag="kt", name=f"kt_{h}")
                if idx < 2:
                    pe_transpose_part(qn, qts[h][:], idx)
                else:
                    pe_transpose_part(kn, kts[h][:], idx - 2)

            def xpose_inputs(h):
                for i in range(4):
                    xpose_part(h, i)

            # head 0 fast-start: half-granule casts so the first transposes
            # and first QK can begin after ~half a tensor has landed
            def prep_head0():
                qn = native_pool.tile([128, S], fp16, tag="nat", name="qn_0")
                kn = native_pool.tile([128, S], fp16, tag="nat", name="kn_0")
                nats[0] = (qn, kn)
                vsb = v_pool.tile([128, SK * 129], fp16, tag="vsb", name="vsb_0")
                vsbs[0] = vsb
                qv = qn[:].rearrange("p (t d) -> p t d", d=D)
                kv = kn[:].rearrange("p (t d) -> p t d", d=D)
                vv = vsb[:].rearrange("p (t c) -> p t c", c=129)
                HT = SK // 2
                for half in range(2):
                    ts = slice(half * HT, (half + 1) * HT)
                    rows = slice(half * (S // 2), (half + 1) * (S // 2))
                    nc.gpsimd.dma_start(
                        qv[:, ts, :], q[0, rows, :].rearrange("(t p) d -> p t d", p=128)
                    )
                    nc.gpsimd.dma_start(
                        kv[:, ts, :], k[0, rows, :].rearrange("(t p) d -> p t d", p=128)
                    )
                    if half == 0:
                        nc.gpsimd.memset(vsb[:], 1.0)
                    nc.gpsimd.dma_start(
                        vv[:, ts, 0:D],
                        v[0, rows, :].rearrange("(t p) d -> p t d", p=128),
                    )
                    xpose_part(0, 0 if half == 0 else 1)  # q half
                    xpose_part(0, 2 if half == 0 else 3)  # k half

            prep_head0()

            for h in range(HPC):
                qt, kt, vsb = qts[h], kts[h], vsbs[h]
                for qc in range(2):  # q-chunks of 1024
                    if qc == 0:
                        cast_inputs(h + 1)
                    qbase = qc * 1024
                    po = [
                        psum_o_pool.tile(
                            [128, 3 * 129], fp32, tag="po", name=f"po_h{h}_{qc}_{i}"
                        )
                        for i in range(3)
                    ]

                    def emit_pv(sk, pt):
                        for sq in range(8):
                            dst = po[sq // 3]
                            off = (sq % 3) * 129
                            # start=True clears the ENTIRE psum bank, so only
                            # the first slice written into each bank may carry
                            # it; the other slices' first writes land on
                            # cleared has_written bits and store rather than
                            # accumulate.
                            nc.tensor.matmul(
                                dst[:, off : off + 129],
                                pt[:, sq * 128 : (sq + 1) * 128],
                                vsb[:, sk * 129 : (sk + 1) * 129],
                                start=(sk == 0 and off == 0),
                                stop=(sk == SK - 1),
                                skip_group_check=True,
                            )

                    # Software pipeline: emit PV(sk-1) after QK(sk) so the
                    # in-order PE has ready work while ACT runs exp(sk).
                    pending = None
                    for sk in range(SK):
                        ps = psum_s_pool.tile([128, 1024], fp32, tag="ps")
                        for j in range(2):
                            # the two MMs land in the tile's two distinct psum
                            # banks, so each may clear (start) its own bank
                            nc.tensor.matmul(
                                ps[:, j * 512 : (j + 1) * 512],
                                kt[:, sk * 128 : (sk + 1) * 128],
                                qt[:, qbase + j * 512 : qbase + (j + 1) * 512],
                                start=True,
                                stop=True,
                            )
                        pt = pt_pool.tile([128, 1024], fp16)
                        nc.scalar.activation(pt[:], ps[:], AF.Exp, scale=SCALE)
                        if qc == 1 and sk in (5, 7, 9, 11):
                            xpose_part(h + 1, (sk - 5) // 2)
                        if pending is not None:
                            emit_pv(*pending)
                        pending = (sk, pt)
                    emit_pv(*pending)

                    for sq in range(8):
                        src = po[sq // 3]
                        off = (sq % 3) * 129
                        r = norm_pool.tile([128, 1], fp32)
                        nc.vector.reciprocal(r[:], src[:, off + D : off + D + 1])
                        ob = out_pool.tile([128, D], fp32)
                        nc.vector.tensor_scalar_mul(ob[:], src[:, off : off + D], r[:])
                        row = qbase + sq * 128
                        nc.sync.dma_start(o[h, row : row + 128, :], ob[:])

    _split_sync_waits(nc, maxw=1)
    return nc


def _get_nc():
    if "nc" not in _CACHE:
        _install_ntff_hook()
        _CACHE["nc"] = _build()
    return _CACHE["nc"]


def run_sharded(query, key, value, trace=False, **trace_kwargs):
    """Run the 8-core SPMD kernel; returns (output [BH,S,D] fp32, results obj)."""
    from concourse.bass_utils import run_bass_kernel_spmd

    nc = _get_nc()
    query = np.ascontiguousarray(np.asarray(query, dtype=np.float32))
    key = np.ascontiguousarray(np.asarray(key, dtype=np.float32))
    value = np.ascontiguousarray(np.asarray(value, dtype=np.float32))
    in_maps = [
        {
            "q": query[c * HPC : (c + 1) * HPC],
            "k": key[c * HPC : (c + 1) * HPC],
            "v": value[c * HPC : (c + 1) * HPC],
        }
        for c in range(N_CORES)
    ]
    res = run_bass_kernel_spmd(
        nc, in_maps, list(range(N_CORES)), trace=trace, **trace_kwargs
    )
    out = np.concatenate([r["o"] for r in res.results], axis=0)
    return out, res


def kernel(key, query, value):
    out, _ = run_sharded(query, key, value, trace=False)
    return out


# revision 30
# speedup vs baseline: 1.0265x; 1.0265x over previous
"""Batched multi-head attention (32 heads, S=2048, D=128, fp32) on 8 Trainium2
NeuronCores.

Sharding: head-parallel — core i computes heads [4i, 4i+4) independently
(no collectives), takes full fp32 inputs, returns the full fp32 output.

Per-core kernel design (4 heads):
  - Q, K are cast fp32->fp16 by a SWDGE (gpsimd) DMA into DRAM staging, then
    xbar DMA-transposed into SBUF as QT/KT [d=128, s=2048] (contraction dim on
    partitions). V is cast-DMA'd natively into SBUF tiles [sk=128, 129] with a
    ones column appended (col 128) so the PV matmul also produces the softmax
    denominator.
  - For each head, for each of 16 sk-tiles:
      * scores^T tile  [sk=128, q=2048] = (K_tile @ Q^T) via 4 matmuls N=512
        (lhsT = KT tile stationary, rhs = QT moving) into PSUM [128,512] slots
      * DVE copies PSUM -> fp32 SBUF staging, one ACT exp over [128, 2048]
        (scale = 1/sqrt(128) folded into the activation's free affine) writes
        P^T fp16. No max-subtraction: scores*scale ~ N(0,1), exp is safe.
      * 16 PV matmuls N=129 (lhsT = P^T slice [128,128] stationary,
        rhs = V_aug tile [128,129] moving) accumulate into 6 packed PSUM
        tiles [128, 387] (3 sq-subtiles per PSUM bank).
  - Normalize: DVE reciprocal of the sums column, tensor_scalar multiply,
    DMA out fp32 [128,128] row-tiles.
"""

import os
import numpy as np

BH, S, D = 32, 2048, 128
N_CORES = 8
HPC = BH // N_CORES  # heads per core
SK = S // 128  # sk tiles per head
SQ = S // 128  # sq subtiles per head
SCALE = 1.0 / float(np.sqrt(D))

_CACHE = {}


def _install_ntff_hook():
    """Provide antenv.axon_hooks (absent in this container) so that
    run_bass_kernel_spmd(trace=True) can capture NTFF profiles."""
    import contextlib, ctypes, sys, types

    if "antenv.axon_hooks" in sys.modules:
        return
    so_path = "/opt/axon/libaxon_pjrt.so"
    hook = None
    try:
        lib = ctypes.CDLL(so_path)
        if hasattr(lib, "axon_start_nrt_profile"):
            lib.axon_start_nrt_profile.argtypes = [
                ctypes.POINTER(ctypes.c_int64),
                ctypes.c_size_t,
            ]
            lib.axon_start_nrt_profile.restype = ctypes.c_int64
            lib.axon_stop_nrt_profile.argtypes = [ctypes.c_char_p]
            lib.axon_stop_nrt_profile.restype = ctypes.c_int64

            @contextlib.contextmanager
            def _h(output_dir, device_ids):
                import jax

                jax.devices()
                if device_ids:
                    ids = (ctypes.c_int64 * len(device_ids))(*device_ids)
                    rc = lib.axon_start_nrt_profile(ids, len(device_ids))
                else:
                    rc = lib.axon_start_nrt_profile(None, 0)
                if rc != 0:
                    raise RuntimeError(f"axon_start_nrt_profile rc={rc}")
                try:
                    yield
                finally:
                    n = lib.axon_stop_nrt_profile(str(output_dir).encode())
                    print(f"ntff profile: {n} file(s) in {output_dir}")

            hook = _h
    except OSError:
        pass
    mod = types.ModuleType("antenv.axon_hooks")
    mod.get_axon_ntff_profile_hook = lambda: hook
    mod.set_axon_ntff_profile_hook = lambda h: None
    sys.modules["antenv.axon_hooks"] = mod


def _split_sync_waits(nc, maxw=1):
    """The walrus codegen in this container rejects instructions carrying more
    than `maxw` sync waits (Tile's scheduler can attach several). Move the
    excess waits onto same-engine nop instructions inserted just before."""
    from concourse import mybir

    n_split = 0
    for f in nc.m.functions:
        for bb in f.blocks:
            out = []
            for inst in bb.instructions:
                si = inst.sync_info
                if si is not None and si.on_wait and len(si.on_wait) > maxw:
                    waits = list(si.on_wait)
                    carriers, keep = waits[:-maxw], waits[-maxw:]
                    si.on_wait = keep
                    inst.sync_info = si
                    for i in range(0, len(carriers), maxw):
                        n_split += 1
                        nop = mybir.InstNoOp(
                            name=f"{inst.name}_wsplit{i}", ins=[], outs=[]
                        )
                        nop.engine = inst.engine
                        nop.sync_info = mybir.SyncInfo(
                            on_wait=carriers[i : i + maxw], on_update=[]
                        )
                        if hasattr(nc, "inst_map"):
                            nc.inst_map[nop.name] = nop
                        out.append(nop)
                out.append(inst)
            bb.instructions[:] = out
    return n_split


def _build():
    import concourse.bass as bass
    from concourse import mybir
    import concourse.tile as tile
    from concourse.masks import make_identity

    fp16 = mybir.dt.float16
    fp32 = mybir.dt.float32
    AF = mybir.ActivationFunctionType

    nc = bass.Bass("TRN2", target_bir_lowering=False, debug=False)
    q = nc.dram_tensor("q", [HPC, S, D], fp32, kind="ExternalInput").ap()
    k = nc.dram_tensor("k", [HPC, S, D], fp32, kind="ExternalInput").ap()
    v = nc.dram_tensor("v", [HPC, S, D], fp32, kind="ExternalInput").ap()
    o = nc.dram_tensor("o", [HPC, S, D], fp32, kind="ExternalOutput").ap()

    with tile.TileContext(nc) as tc:
        with (
            tc.tile_pool(name="ident", bufs=1) as ident_pool,
            tc.tile_pool(name="native", bufs=4) as native_pool,
            tc.tile_pool(name="qt", bufs=HPC) as qt_pool,
            tc.tile_pool(name="kt", bufs=HPC) as kt_pool,
            tc.tile_pool(name="vsb", bufs=HPC) as v_pool,
            tc.tile_pool(name="pt", bufs=3) as pt_pool,
            tc.tile_pool(name="psum_s", bufs=2, space="PSUM") as psum_s_pool,
            tc.tile_pool(name="psum_o", bufs=4, space="PSUM") as psum_o_pool,
            tc.tile_pool(name="outsb", bufs=3) as out_pool,
            tc.tile_pool(name="norm", bufs=4) as norm_pool,
        ):
            # ---- input prep: NO DMA-transposes (Tile globally serializes
            # every DMA against any in-flight xbar transpose, which makes
            # input prep a ~110us serial chain). Instead: SWDGE cast-DMA to
            # SBUF in the native [s,d] block layout, then transpose each
            # [128,128] block on the PE with a plain matmul against an fp16
            # identity, DVE-copying PSUM -> qt/kt. Transposes borrow psum_s
            # slots; for head h+1 they are emitted mid-way through head h's
            # second chunk so their slot claims sit behind the critical
            # QK/exp traffic but complete before head h+1 begins.
            ident = ident_pool.tile([128, 128], fp16)
            make_identity(nc, ident[:])

            qts, kts, vsbs, nats = {}, {}, {}, {}

            def cast_inputs(h):
                if h >= HPC:
                    return
                qn = native_pool.tile([128, S], fp16, tag="nat", name=f"qn_{h}")
                nc.gpsimd.dma_start(
                    qn[:].rearrange("p (t d) -> p t d", d=D),
                    q[h].rearrange("(t p) d -> p t d", p=128),
                )
                kn = native_pool.tile([128, S], fp16, tag="nat", name=f"kn_{h}")
                nc.gpsimd.dma_start(
                    kn[:].rearrange("p (t d) -> p t d", d=D),
                    k[h].rearrange("(t p) d -> p t d", p=128),
                )
                nats[h] = (qn, kn)
                vsb = v_pool.tile([128, SK * 129], fp16, tag="vsb", name=f"vsb_{h}")
                nc.gpsimd.memset(vsb[:], 1.0)
                vv = vsb[:].rearrange("p (t c) -> p t c", c=129)
                nc.gpsimd.dma_start(
                    vv[:, :, 0:D], v[h].rearrange("(t p) d -> p t d", p=128)
                )
                vsbs[h] = vsb

            def pe_transpose_part(nat, out, g):
                slot = psum_s_pool.tile([128, 1024], fp32, tag="ps")
                for t in range(8):
                    blk = (g * 8 + t) * 128
                    nc.tensor.matmul(
                        slot[:, t * 128 : (t + 1) * 128],
                        nat[:, blk : blk + 128],
                        ident[:],
                        start=(t % 4 == 0),
                        stop=True,
                        skip_group_check=True,
                    )
                nc.vector.tensor_copy(out[:, g * 1024 : (g + 1) * 1024], slot[:])

            def xpose_part(h, idx):
                """idx 0,1 -> q halves; 2,3 -> k halves. One psum slot-use
                each, so the bursts can be spread between QK/exp traffic."""
                if h >= HPC:
                    return
                qn, kn = nats[h]
                if idx == 0:
                    qts[h] = qt_pool.tile([128, S], fp16, tag="qt", name=f"qt_{h}")
                if idx == 2:
                    kts[h] = kt_pool.tile([128, S], fp16, tag="kt", name=f"kt_{h}")
                if idx < 2:
                    pe_transpose_part(qn, qts[h][:], idx)
                else:
                    pe_transpose_part(kn, kts[h][:], idx - 2)

            def xpose_inputs(h):
                for i in range(4):
                    xpose_part(h, i)

            # head 0 fast-start: half-granule casts so the first transposes
            # and first QK can begin after ~half a tensor has landed
            def prep_head0():
                qn = native_pool.tile([128, S], fp16, tag="nat", name="qn_0")
                kn = native_pool.tile([128, S], fp16, tag="nat", name="kn_0")
                nats[0] = (qn, kn)
                vsb = v_pool.tile([128, SK * 129], fp16, tag="vsb", name="vsb_0")
                vsbs[0] = vsb
                qv = qn[:].rearrange("p (t d) -> p t d", d=D)
                kv = kn[:].rearrange("p (t d) -> p t d", d=D)
                vv = vsb[:].rearrange("p (t c) -> p t c", c=129)
                HT = SK // 2
                for half in range(2):
                    ts = slice(half * HT, (half + 1) * HT)
                    rows = slice(half * (S // 2), (half + 1) * (S // 2))
                    nc.gpsimd.dma_start(
                        qv[:, ts, :], q[0, rows, :].rearrange("(t p) d -> p t d", p=128)
                    )
                    nc.gpsimd.dma_start(
                        kv[:, ts, :], k[0, rows, :].rearrange("(t p) d -> p t d", p=128)
                    )
                    if half == 0:
                        nc.gpsimd.memset(vsb[:], 1.0)
                    nc.gpsimd.dma_start(
                        vv[:, ts, 0:D],
                        v[0, rows, :].rearrange("(t p) d -> p t d", p=128),
                    )
                    xpose_part(0, 0 if half == 0 else 1)  # q half
                    xpose_part(0, 2 if half == 0 else 3)  # k half

            prep_head0()

            for h in range(HPC):
                qt, kt, vsb = qts[h], kts[h], vsbs[h]
                for qc in range(2):  # q-chunks of 1024
                    if qc == 0:
                        cast_inputs(h + 1)
                    qbase = qc * 1024
                    po = [
                        psum_o_pool.tile(
                            [128, 3 * 129], fp32, tag="po", name=f"po_h{h}_{qc}_{i}"
                        )
                        for i in range(3)
                    ]

                    def emit_pv(sk, pt):
                        for sq in range(8):
                            dst = po[sq // 3]
                            off = (sq % 3) * 129
                            # start=True clears the ENTIRE psum bank, so only
                            # the first slice written into each bank may carry
                            # it; the other slices' first writes land on
                            # cleared has_written bits and store rather than
                            # accumulate.
                            nc.tensor.matmul(
                                dst[:, off : off + 129],
                                pt[:, sq * 128 : (sq + 1) * 128],
                                vsb[:, sk * 129 : (sk + 1) * 129],
                                start=(sk == 0 and off == 0),
                                stop=(sk == SK - 1),
                                skip_group_check=True,
                            )

                    # Software pipeline: emit PV(sk-1) after QK(sk) so the
                    # in-order PE has ready work while ACT runs exp(sk).
                    pending = None
                    for sk in range(SK):
                        ps = psum_s_pool.tile([128, 1024], fp32, tag="ps")
                        for j in range(2):
                            # the two MMs land in the tile's two distinct psum
                            # banks, so each may clear (start) its own bank
                            nc.tensor.matmul(
                                ps[:, j * 512 : (j + 1) * 512],
                                kt[:, sk * 128 : (sk + 1) * 128],
                                qt[:, qbase + j * 512 : qbase + (j + 1) * 512],
                                start=True,
                                stop=True,
                            )
                        pt = pt_pool.tile([128, 1024], fp16)
                        nc.scalar.activation(pt[:], ps[:], AF.Exp, scale=SCALE)
                        if qc == 1 and sk in (5, 7, 9, 11):
                            xpose_part(h + 1, (sk - 5) // 2)
                        if pending is not None:
                            emit_pv(*pending)
                        pending = (sk, pt)
                    emit_pv(*pending)

                    ob = out_pool.tile([128, 8 * D], fp32)
                    for sq in range(8):
                        src = po[sq // 3]
                        off = (sq % 3) * 129
                        r = norm_pool.tile([128, 1], fp32)
                        nc.vector.reciprocal(r[:], src[:, off + D : off + D + 1])
                        nc.vector.tensor_scalar_mul(
                            ob[:, sq * D : (sq + 1) * D], src[:, off : off + D], r[:]
                        )
                    # one 512KB store for the whole chunk: ob is [p, (sq d)],
                    # DRAM rows are qbase + sq*128 + p
                    nc.sync.dma_start(
                        o[h, qbase : qbase + 1024, :].rearrange(
                            "(t p) d -> p t d", p=128
                        ),
                        ob[:].rearrange("p (t d) -> p t d", d=D),
                    )

    _split_sync_waits(nc, maxw=1)
    return nc


def _get_nc():
    if "nc" not in _CACHE:
        _install_ntff_hook()
        _CACHE["nc"] = _build()
    return _CACHE["nc"]


def run_sharded(query, key, value, trace=False, **trace_kwargs):
    """Run the 8-core SPMD kernel; returns (output [BH,S,D] fp32, results obj)."""
    from concourse.bass_utils import run_bass_kernel_spmd

    nc = _get_nc()
    query = np.ascontiguousarray(np.asarray(query, dtype=np.float32))
    key = np.ascontiguousarray(np.asarray(key, dtype=np.float32))
    value = np.ascontiguousarray(np.asarray(value, dtype=np.float32))
    in_maps = [
        {
            "q": query[c * HPC : (c + 1) * HPC],
            "k": key[c * HPC : (c + 1) * HPC],
            "v": value[c * HPC : (c + 1) * HPC],
        }
        for c in range(N_CORES)
    ]
    res = run_bass_kernel_spmd(
        nc, in_maps, list(range(N_CORES)), trace=trace, **trace_kwargs
    )
    out = np.concatenate([r["o"] for r in res.results], axis=0)
    return out, res


def kernel(key, query, value):
    out, _ = run_sharded(query, key, value, trace=False)
    return out
